# revision 1
# baseline (speedup 1.0000x reference)
"""Trainium2 Bass kernel for nn_BAFM_BRB_65249143161427 (segment_reduce).

Reference semantics: downsample x0/x1 by 8 (nearest), compute directional
running segment means between zero-boundaries of x1 along rows and columns,
sum the 4 directional terms, upsample by 8.

Sharding: pure data parallel — image n -> core n (N=8, 8 cores).
Each core processes one full 2048x2048 image.

Structure (per core): the 16 MB output store stream dominates, so the
program is ordered to keep the DMA engines saturated end to end: loads
issue on one queue in critical-first order (everything store-block (0,0)
needs, then deferred right halves that drain while the first block's
compute chain runs), the counts/scans/elementwise work is spread across
DVE/Pool/ACT with scan ops kept off Pool (scans lower to TensorScalarPtr,
which Pool's codegen rejects), and every load/store spans the full 128
partitions (partition-offset DMAs corrupted data on HW). The remaining
passes' engine work hides under the ~47 us store stream; tile_wait_until
gates keep the greedy scheduler from slotting slack work into the
critical window.
"""
import sys

sys.path.insert(0, "/opt/trn_rl_repo")

import numpy as np

H = W = 2048
S = 8
HD, WD = H // S, W // S      # 256 x 256 downsampled grid
P = 128                      # SBUF partitions
NT = HD // P                 # 2 row-tiles of the downsampled grid
N_CORES = 8

_CACHE = {}


def _revap(ap2d):
    """Reverse the last (free) dim of a 2D access pattern."""
    from concourse.ap import AP

    pairs = [list(p) for p in ap2d.ap]
    step, n = pairs[-1]
    return AP(ap2d.tensor, ap2d.offset + (n - 1) * step, pairs[:-1] + [[-step, n]])


def _bcast8(ap2d):
    """Append a step-0 count-8 inner dim (each element replicated 8x)."""
    from concourse.ap import AP

    pairs = [list(p) for p in ap2d.ap]
    return AP(ap2d.tensor, ap2d.offset, pairs + [[0, 8]])


def build_program(loop_n: int = 1, fast_recip=True, pool_elem=True,
                  unroll=True):
    import concourse.bacc as bacc
    import concourse.tile as tile
    from concourse import mybir
    from concourse.masks import make_identity
    from contextlib import ExitStack

    f32 = mybir.dt.float32
    i32 = mybir.dt.int32
    MUL = mybir.AluOpType.mult
    ADD = mybir.AluOpType.add
    NE = mybir.AluOpType.not_equal

    # Bacc (not raw Bass): its compile() splits multi-wait sync commands,
    # which TRN2 engines (1 wait/instruction) require.
    nc = bacc.Bacc("TRN2")
    x0 = nc.declare_dram_parameter("x0", [H, W], f32, isOutput=False)
    x1 = nc.declare_dram_parameter("x1", [H, W], i32, isOutput=False)
    y = nc.declare_dram_parameter("y", [H, W], f32, isOutput=True)

    # DRAM viewed with rows grouped by 8: [256, 8, 2048]
    x0g = x0[:].rearrange("(r e) w -> r e w", e=8)
    x1g = x1[:].rearrange("(r e) w -> r e w", e=8)
    yg = y[:].rearrange("(r e) w -> r e w", e=8)

    with tile.TileContext(nc) as tc:
        with ExitStack() as ctx:
            const_pool = ctx.enter_context(tc.tile_pool(name="const", bufs=1))
            io_pool = ctx.enter_context(tc.tile_pool(name="io", bufs=1))
            work = ctx.enter_context(tc.tile_pool(name="work", bufs=1))
            # PSUM is fully subscribed: this pool (2 tags x 2 bufs = 4
            # banks) + psum_x (2 tags x 2 bufs = 4 banks) = all 8 banks
            psum = ctx.enter_context(
                tc.tile_pool(name="psum", bufs=2, space="PSUM")
            )
            psum_x = ctx.enter_context(
                tc.tile_pool(name="psum_x", bufs=2, space="PSUM")
            )

            ident_g = const_pool.tile([P, P], f32)
            make_identity(nc, ident_g[:])
            ident = const_pool.tile([P, P], f32)
            nc.vector.tensor_copy(ident[:], ident_g[:])
            ones = const_pool.tile([P, WD], f32)
            nc.gpsimd.memset(ones[:], 1.0)
            elem_eng = nc.gpsimd if pool_elem else nc.vector

            def body(bufi=None):
                # ---- tiles ----
                # bufi selects the I/O buffer set: in the unrolled timing
                # loop, alternating sets let iteration k+1's loads and
                # expands proceed while iteration k's stores still read the
                # other set (no cross-iteration WAR on x1r/x0r/ye). bufi
                # None (the single-shot graded program) keeps the original
                # unsuffixed tile names so that program is unchanged.
                sfx = "" if bufi is None else f"b{bufi}"
                am = [work.tile([P, WD], f32, tag=f"am{t}", name=f"am{t}") for t in range(NT)]
                # transposed x stays in PSUM (scans read it directly);
                # only the transposed mask is staged to SBUF
                xT = [psum_x.tile([P, HD], f32, tag=f"xT{c}", name=f"xT{c}") for c in range(NT)]
                aT = [work.tile([P, HD], f32, tag=f"aT{c}", name=f"aT{c}") for c in range(NT)]
                yh = [None] * NT    # horizontal-pass results
                yv = [None] * NT    # vertical-pass results (transposed)
                ye = [io_pool.tile([P, W], f32, tag=f"ye{t}{sfx}",
                                   name=f"ye{t}{sfx}") for t in range(NT)]

                x1r = [None] * NT
                x0r = [None] * NT

                def load_piece(src, dst, t, r0, r1, lo, hi):
                    nc.sync.dma_start(
                        out=dst[t][r0:r1, lo:hi],
                        in_=src[t * P + r0:t * P + r1, 0, lo:hi],
                    )

                def load_x1(t, r0, r1, lo, hi):
                    if x1r[t] is None:
                        x1r[t] = io_pool.tile(
                            [P, W], i32, tag=f"x1r{t}{sfx}",
                            name=f"x1r{t}{sfx}")
                    load_piece(x1g, x1r, t, r0, r1, lo, hi)

                def load_x0(t, r0, r1, lo, hi):
                    # row/column-piece DMA; consumers read the row tile
                    # directly through stride-8 views (no downsample copy, no
                    # extra DMA-sem hop on the critical path)
                    if x0r[t] is None:
                        x0r[t] = io_pool.tile(
                            [P, W], f32, tag=f"x0r{t}{sfx}",
                            name=f"x0r{t}{sfx}")
                    load_piece(x0g, x0r, t, r0, r1, lo, hi)

                def mask_x1(t, r0, r1, lo, hi):
                    # (DVE: Pool rejects TensorScalarPtr in codegen)
                    nc.vector.tensor_scalar(
                        out=am[t][r0:r1, lo:hi],
                        in0=x1r[t][r0:r1, lo * 8:hi * 8].rearrange(
                            "p (a b) -> p a b", b=8)[:, :, 0],
                        scalar1=0, scalar2=None, op0=NE,
                    )

                def xdv(t, lo=0, hi=WD, r0=0, r1=P):
                    """Stride-8 view of x0r[t] covering downsampled cols
                    [lo, hi) of rows [r0, r1)."""
                    return x0r[t][r0:r1, lo * 8:hi * 8].rearrange(
                        "p (a b) -> p a b", b=8)[:, :, 0]

                def seg_counts(at, pf, r0=0, r1=P):
                    """Mask-only stage: count reciprocals + invalid mask q
                    (counts_div + counts_q in one go, for the slack-side
                    passes)."""
                    st = counts_div(at, pf, r0, r1)
                    return counts_q(st, pf, r0, r1)

                def counts_div(at, pf, r0=0, r1=P):
                    """Count scans + divisor reciprocals (all DVE — only
                    baseline-proven op forms)."""
                    c_lr = work.tile([P, WD], f32, tag=f"clr{pf}")
                    c_rl = work.tile([P, WD], f32, tag=f"crl{pf}")  # reversed
                    a = at[r0:r1, :]
                    a_r = _revap(a)
                    ttscan = nc.vector.tensor_tensor_scan
                    ttscan(c_lr[r0:r1, :], a, ones[r0:r1, :], 0.0, MUL, ADD)
                    ttscan(c_rl[r0:r1, :], a_r, ones[r0:r1, :], 0.0, MUL, ADD)
                    ilr = work.tile([P, WD], f32, tag=f"ilr{pf}")
                    irl = work.tile([P, WD], f32, tag=f"irl{pf}")
                    if fast_recip:
                        nc.vector.reciprocal_approx_fast(
                            ilr[r0:r1, :], c_lr[r0:r1, :])
                        nc.vector.reciprocal_approx_fast(
                            irl[r0:r1, :], c_rl[r0:r1, :])
                    else:
                        scr = work.tile([P, WD], f32, tag=f"scr{pf}")
                        nc.vector.reciprocal_approx_accurate(
                            ilr[r0:r1, :], c_lr[r0:r1, :], scr[r0:r1, :])
                        nc.vector.reciprocal_approx_accurate(
                            irl[r0:r1, :], c_rl[r0:r1, :], scr[r0:r1, :])
                    return dict(a=a, a_r=a_r, ilr=ilr, irl=irl)

                def counts_q(st, pf, r0=0, r1=P):
                    """Invalid-lane mask (baseline-proven form): nb/na mask
                    product scans on DVE; q = nb + rev(na) as an i32 DVE
                    TensorTensor — nonzero where an enclosing boundary is
                    missing on either side."""
                    nb = work.tile([P, WD], f32, tag=f"nb{pf}")
                    na = work.tile([P, WD], f32, tag=f"na{pf}")  # reversed
                    ttscan = nc.vector.tensor_tensor_scan
                    ttscan(nb[r0:r1, :], st["a"], st["a"], 1.0, MUL, MUL)
                    ttscan(na[r0:r1, :], st["a_r"], st["a_r"], 1.0, MUL, MUL)
                    q = work.tile([P, WD], i32, tag=f"q{pf}")
                    nc.vector.tensor_tensor(
                        q[r0:r1, :], nb[r0:r1, :], _revap(na[r0:r1, :]), ADD)
                    return st["ilr"], st["irl"], q

                def two_x_calc(two_x, xt, r0, r1, on_dve):
                    # ACT's queue is captive to store-DMA descriptor issue
                    # for long stretches; passes whose fallback is needed
                    # while that happens compute 2x on DVE instead
                    if on_dve:
                        nc.vector.tensor_scalar(
                            out=two_x[r0:r1, :], in0=xt, scalar1=2.0,
                            scalar2=None, op0=MUL)
                    else:
                        nc.scalar.mul(two_x[r0:r1, :], xt, 2.0)

                def seg_sums(xt, at, cnts, pf, r0=0, r1=P, tx_dve=False):
                    """Value stage: segment sums -> means -> m with fallback."""
                    ilr, irl, q = cnts
                    s_lr = work.tile([P, WD], f32, tag=f"slr{pf}")
                    s_rl = work.tile([P, WD], f32, tag=f"srl{pf}")  # reversed
                    a = at[r0:r1, :]
                    ttscan = nc.vector.tensor_tensor_scan
                    ttscan(s_lr[r0:r1, :], a, xt, 0.0, MUL, ADD)
                    ttscan(s_rl[r0:r1, :], _revap(a), _revap(xt), 0.0, MUL, ADD)
                    elem_eng.tensor_tensor(
                        s_lr[r0:r1, :], s_lr[r0:r1, :], ilr[r0:r1, :], MUL)
                    elem_eng.tensor_tensor(
                        s_rl[r0:r1, :], s_rl[r0:r1, :], irl[r0:r1, :], MUL)
                    m = work.tile([P, WD], f32, tag=f"m{pf}")
                    two_x = work.tile([P, WD], f32, tag=f"tx{pf}")
                    two_x_calc(two_x, xt, r0, r1, tx_dve)
                    nc.vector.tensor_tensor(
                        m[r0:r1, :], s_lr[r0:r1, :], _revap(s_rl[r0:r1, :]),
                        ADD)
                    nc.vector.copy_predicated(
                        m[r0:r1, :], q[r0:r1, :], two_x[r0:r1, :])
                    return m

                def seg_scans(xt, at, pf, r0=0, r1=P, tx_dve=False):
                    """Scan stage of the critical-path passes: the two
                    directional sum scans (DVE-only: scans lower to
                    TensorScalarPtr) plus the 2x fallback. Issued separately
                    from the half-finishes so the engine queues order all
                    scans before any elementwise tail."""
                    s_lr = work.tile([P, WD], f32, tag=f"slr{pf}")
                    s_rl = work.tile([P, WD], f32, tag=f"srl{pf}")  # reversed
                    a = at[r0:r1, :]
                    nc.vector.tensor_tensor_scan(
                        s_lr[r0:r1, :], a, xt, 0.0, MUL, ADD)
                    nc.vector.tensor_tensor_scan(
                        s_rl[r0:r1, :], _revap(a), _revap(xt), 0.0, MUL, ADD)
                    m = work.tile([P, WD], f32, tag=f"m{pf}")
                    two_x = work.tile([P, WD], f32, tag=f"tx{pf}")
                    two_x_calc(two_x, xt, r0, r1, tx_dve)
                    return dict(s_lr=s_lr, s_rl=s_rl, m=m, two_x=two_x)

                def seg_half(st, cnts, h, r0=0, r1=P,
                             eng=None, eng2=None, eng3=None):
                    """Elementwise tail for column-half h of a scanned pass:
                    means, combine, predicated 2x fallback."""
                    ilr, irl, q = cnts
                    s_lr, s_rl, m, two_x = (
                        st["s_lr"], st["s_rl"], st["m"], st["two_x"])
                    hd_ = WD // 2
                    lo, hi = h * hd_, (h + 1) * hd_
                    rlo, rhi = WD - hi, WD - lo  # mirrored slice (rev space)
                    (eng or nc.vector).tensor_tensor(
                        s_lr[r0:r1, lo:hi], s_lr[r0:r1, lo:hi],
                        ilr[r0:r1, lo:hi], MUL)
                    (eng2 or eng or nc.vector).tensor_tensor(
                        s_rl[r0:r1, rlo:rhi], s_rl[r0:r1, rlo:rhi],
                        irl[r0:r1, rlo:rhi], MUL)
                    (eng3 or nc.vector).tensor_tensor(
                        m[r0:r1, lo:hi], s_lr[r0:r1, lo:hi],
                        _revap(s_rl[r0:r1, rlo:rhi]), ADD)
                    nc.vector.copy_predicated(
                        m[r0:r1, lo:hi], q[r0:r1, lo:hi],
                        two_x[r0:r1, lo:hi])

                def transpose_a(c):
                    """Transposed mask -> aT[c] (SBUF, bounced via PSUM:
                    scan data0 and data1 cannot both live in PSUM)."""
                    for t in range(NT):
                        pb = psum.tile([P, P], f32, tag="ptr")
                        nc.tensor.transpose(
                            pb[:], am[t][:, c * P:(c + 1) * P], ident[:]
                        )
                        nc.scalar.copy(aT[c][:, t * P:(t + 1) * P], pb[:])

                def transpose_x(c):
                    """Transpose x straight into the PSUM tile the vertical
                    sum-scans read."""
                    for t in range(NT):
                        nc.tensor.transpose(
                            xT[c][:, t * P:(t + 1) * P],
                            xdv(t, c * P, (c + 1) * P), ident[:],
                        )

                pbs = {}

                def combine_store(t, c, r0=0, r1=P, first=False, m=None):
                    """y block rows [r0,r1) of (t,c) = m[r,cP:] +
                    yv[c][:,tP+r]^T; expand 8x8, store 8 row-replicas.
                    The yv transpose is done once per (t,c) and sliced per
                    row group. `m` must be the EXACT tile object the h-pass
                    wrote (tile pool tags alias memory across tile() calls
                    but do NOT link deps between the logical tensors).
                    first=True keeps the expansion on DVE (skips the ACT hop
                    on the path that opens the store stream)."""
                    if m is None:
                        m = yh[t]
                    if (t, c) not in pbs:
                        pb = psum.tile([P, P], f32, tag="ptb")
                        nc.tensor.transpose(
                            pb[:], yv[c][:, t * P:(t + 1) * P], ident[:]
                        )
                        pbs[(t, c)] = pb
                    pb = pbs[(t, c)]
                    cw = W // NT
                    ye_view = ye[t][r0:r1, c * cw:(c + 1) * cw].rearrange(
                        "p (a b) -> p a b", b=8)
                    # (a fused bcast-input tensor_tensor straight into
                    # ye_view costs 1192ns on DVE — slower than the 258+594
                    # two-step)
                    ysum = work.tile([P, P], f32, tag=f"ys{t}{c}")
                    nc.vector.tensor_tensor(
                        ysum[r0:r1, :], m[r0:r1, c * P:(c + 1) * P],
                        pb[r0:r1, :], ADD
                    )
                    # DVE bcast-copy (594ns) beats the ACT hop (1038ns), and
                    # ACT's queue is captive to store-DMA descriptor issue
                    # while the stream runs; DVE is idle by the time each
                    # later block's expand is due
                    nc.vector.tensor_copy(ye_view, _bcast8(ysum[r0:r1, :]))
                    # 8 parallel row-replica DMAs: on real HW multiple
                    # in-flight DMAs fan out across queues (a single
                    # broadcast-source DMA measured ~20us slower)
                    # alternate HWDGE issuers (SP / ACT) so descriptor
                    # generation for the store stream runs on two sequencers
                    for k in range(8):
                        issuer = nc.sync if k % 2 == 0 else nc.scalar
                        issuer.dma_start(
                            out=yg[t * P + r0:t * P + r1, k,
                                   c * cw:(c + 1) * cw],
                            in_=ye[t][r0:r1, c * cw:(c + 1) * cw],
                        )

                # ---- ordered for earliest store start ----
                # The first store block is the TOP-LEFT 64-row group (rows
                # 0..63 x cols 0..127): its critical loads are 2.5MB (x1/x0
                # t0-top full width for the h-pass + x1/x0 left halves of
                # all rows for the v-pass). Loads issue on one queue in
                # DMA-priority order with that set first; the four deferred
                # right-half pieces drain while the first block's chain runs
                # so the DMA engines never idle between loads and stores.
                # x1 t0-top goes first so the h-count scans run early, off
                # the critical path; the h sum-scans gate on the last
                # critical piece (x0 t0-top right).
                # v6: no partition-offset DMAs — every load/store spans the
                # full 128 partitions; only column ranges vary.
                HW2 = W // 2
                load_x1(0, 0, P, 0, HW2)       # t0 left (h+v masks)
                load_x1(0, 0, P, HW2, W)       # t0 right (h masks)
                load_x1(1, 0, P, 0, HW2)       # t1 left (v masks)
                load_x0(1, 0, P, 0, HW2)       # t1 left (v scans)
                load_x0(0, 0, P, 0, HW2)       # t0 left (v+h)
                load_x0(0, 0, P, HW2, W)       # t0 right: h gate
                load_x1(1, 0, P, HW2, W)       # deferred (h1)
                load_x0(1, 0, P, HW2, W)       # deferred (h1/v1)
                mask_x1(0, 0, P, 0, WD // 2)
                mask_x1(0, 0, P, WD // 2, WD)
                mask_x1(1, 0, P, 0, WD // 2)
                ch0d = counts_div(am[0], "h0")  # early, off-critical
                transpose_x(0)
                transpose_a(0)
                cv0d = counts_div(aT[0], "v0")
                cv0 = counts_q(cv0d, "v0")
                ch0 = counts_q(ch0d, "h0")
                # program order == scheduler priority here: loads < critical
                # chain < deferred sections, which is exactly the preference
                # order (an explicit high_priority block would renumber the
                # first stores BELOW the deferred loads and reorder the SP
                # queue against them)
                st_v0 = seg_scans(xT[0][:], aT[0], "v0")
                st_h0 = seg_scans(xdv(0), am[0], "h0")
                seg_half(st_v0, cv0, 0, eng=nc.gpsimd, eng2=nc.gpsimd)
                seg_half(st_h0, ch0, 0, eng=nc.gpsimd, eng2=nc.gpsimd)
                yv[0] = st_v0["m"]
                yh[0] = st_h0["m"]
                combine_store(0, 0, first=True)  # stream opens
                # Everything below has large slack (the store stream runs
                # for ~46us); tile_wait_until keeps the greedy scheduler
                # from slotting it into engine-idle moments BEFORE the
                # critical chain's DMA deps land, which would push the
                # first store out.
                with tc.tile_wait_until(0.018):
                    mask_x1(1, 0, P, WD // 2, WD)
                    seg_half(st_v0, cv0, 1)
                    seg_half(st_h0, ch0, 1)
                    ch1 = seg_counts(am[1], "h1")
                    yh[1] = seg_sums(xdv(1), am[1], ch1, "h1", tx_dve=True)
                    combine_store(1, 0)
                with tc.tile_wait_until(0.022):
                    transpose_a(1)
                    cv1 = seg_counts(aT[1], "v1")
                    transpose_x(1)
                    yv[1] = seg_sums(xT[1][:], aT[1], cv1, "v1")
                    combine_store(0, 1)
                    combine_store(1, 1)

            if loop_n > 1 and unroll:
                # 2x-unrolled timing loop with alternating I/O buffer sets;
                # an odd remainder iteration runs after the loop so the
                # total iteration count is exactly loop_n
                with tc.For_i(0, loop_n // 2, 1):
                    body(0)
                    body(1)
                if loop_n % 2:
                    body(0)
            elif loop_n > 1:
                with tc.For_i(0, loop_n, 1):
                    body()
            else:
                body()

    nc.compile()
    return nc


def _get_nc():
    if "nc" not in _CACHE:
        _CACHE["nc"] = build_program()
    return _CACHE["nc"]


def kernel(x0: np.ndarray, x1: np.ndarray) -> np.ndarray:
    from concourse.bass_utils import run_bass_kernel_spmd

    nc = _get_nc()
    n = x0.shape[0]
    in_maps = [
        {"x0": np.ascontiguousarray(x0[i, 0]),
         "x1": np.ascontiguousarray(x1[i, 0])}
        for i in range(n)
    ]
    res = run_bass_kernel_spmd(nc, in_maps, list(range(N_CORES)))
    out = np.stack([res.results[i]["y"] for i in range(n)])
    return out.reshape(n, 1, H, W).astype(np.float32)

